# revision 1
# baseline (speedup 1.0000x reference)
"""Trainium2 Bass kernel for nn_ODE4: explicit-Euler neural ODE + MLP head.

  y_{t+1} = y_t + dt_t * (tanh([y_t, e_t] @ Wr1 + br1) @ Wr2 + br2)
  out     = relu(preds @ W1 + b1) @ W2 + b2          # preds = [y_0..y_{T-1}]

Sharding: pure data parallel over batch B across 8 cores (128 rows each);
tiny weights replicated; the sequential scan over T stays local per core.

The scan runs in PRE-ACTIVATION space: the state p_t = Wy^T y_t + We^T e_t
+ br1 lives in a persistent fp32 PSUM accumulator, so one step is only

    h_t = fp16(tanh(p_t))                                (ACT, psum->sbuf)
    p  += Lfull_t^T @ [h_t; h_{t-1}; 1; d-pairs]         (ONE fp16 matmul)

i.e. the serial dependency chain is two hops (ACT -> PE -> ACT),
~712 ns/step on HW. Numerical scheme (end-to-end rel err ~1e-3):
  * d_t = e_{t+1}-e_t is host-split into an fp16 pair (exact to 2^-22);
    weight-rounding errors on the d-path then telescope (bounded).
  * the fp16 rounding residual of the per-step weight block is applied
    through DUPLICATED rhs rows: the d-rows are host-duplicated in the
    DMA stream, and h_{t-1} is replicated into rows 32:64 of the next
    slot by an off-chain DVE partition-offset copy, so the h-residual
    correction rides in the same matmul one step late (validated vs a
    bit-accurate numpy emulation, sim_numerics.py).
  * the readout head is a SECOND tiny PSUM accumulator: pre1_t accumulates
    via hm_t = Lfull_t @ M1^T (M1 = W1^T pinv(Wy^T); the exogenous terms
    cancel exactly), relu-snapshotted to fp16 by DVE tensor_scalar_max on
    a lag-tolerant side chain, then contracted with W2 into a batch-major
    [128, 2*TC] PSUM tile for a clean output DMA.

x never touches the device: the host ships d-pairs and the folded
per-step weight streams (lm/hm) as packed fp16 DMA streams.
"""

import numpy as np
from contextlib import ExitStack

import concourse.bacc as bacc
import concourse.mybir as mybir
from concourse.tile import TileContext
from concourse import bass_utils

F32 = mybir.dt.float32
AF = mybir.ActivationFunctionType

B, T, S, E, H = 1024, 4096, 8, 8, 32
NCORES = 8
BC = B // NCORES  # 128 per-core batch rows = matmul free dim


K_RHS = 49   # legacy 49-row stack layout used to build the exact
             # per-step weights before the v6 re-layout

# ---------------------------------------------------------------------------
# v6: v5 with the fine-correction matmul MERGED into the main matmul.
#
# The rhs stack widens to 98 rows: [h_t; h_{t-1}; d-parts; d-parts-dup].
# h_{t-1} is replicated into rows 32:64 of the NEXT slot by an off-chain
# DVE partition-offset copy (validated on HW); the d-parts are host-
# duplicated in the DMA stream. The combined lhsT carries main weights
# plus the fp16 rounding residuals (h-residual one step lagged, d/bias
# residuals current) -- one matmul per step updates the scan state with
# full fine correction. Serial chain: TANH -> single MM.
# ---------------------------------------------------------------------------

K2 = 98  # [h 32; h_lag 32; ones 1; d_c 8; d_f 8; d_c' 8; d_f' 8; ones' 1]
KH = 65  # head matmul contracts [h; h_lag(zero-weighted); ones]
KM = 90  # scan matmul contracts rows 0:90 only -- drops the We_f*d_f
         # second-order residual rows and the zero bias-residual row
         # (both O(1e-7) per step), shaving PE array fill


def build_ode_nc_v6(T=T, TC=64):
    assert TC % 4 == 0 and T % TC == 0
    nchunks = T // TC
    F16 = mybir.dt.float16

    nc = bacc.Bacc()
    rhsd_d = nc.dram_tensor("rhsd", [34, T * 128], F16, kind="ExternalInput")
    lm_d = nc.dram_tensor("lm", [K2, T * H], F16, kind="ExternalInput")
    hm_d = nc.dram_tensor("hm", [KH, T * 10], F16, kind="ExternalInput")
    w2_d = nc.dram_tensor("w2f", [10, 2], F16, kind="ExternalInput")
    p0_d = nc.dram_tensor("p0t", [H, BC], F32, kind="ExternalInput")
    h0_d = nc.dram_tensor("pre10", [10, BC], F32, kind="ExternalInput")
    id_d = nc.dram_tensor("id32", [H, H], F32, kind="ExternalInput")
    idh_d = nc.dram_tensor("id10", [10, 10], F32, kind="ExternalInput")
    out_d = nc.dram_tensor("out", [BC, T * 2], F32, kind="ExternalOutput")

    with TileContext(nc) as tc, ExitStack() as ctx:
        cpool = ctx.enter_context(tc.tile_pool(name="consts", bufs=1))
        rhsp = ctx.enter_context(tc.tile_pool(name="rhs", bufs=3))
        lmp = ctx.enter_context(tc.tile_pool(name="lm", bufs=3))
        hmp = ctx.enter_context(tc.tile_pool(name="hm", bufs=3))
        usp = ctx.enter_context(tc.tile_pool(name="u", bufs=3))
        osbp = ctx.enter_context(tc.tile_pool(name="osb", bufs=2))
        ppp = ctx.enter_context(tc.tile_pool(name="ppp", bufs=1, space="PSUM"))
        pop = ctx.enter_context(tc.tile_pool(name="pop", bufs=2, space="PSUM"))

        def cload(name, shape, dram, dt_=F16):
            t_ = cpool.tile(shape, dt_, tag=name)
            nc.sync.dma_start(t_[:], dram[:])
            return t_

        w2_t = cload("w2", [10, 2], w2_d)
        p0_t = cload("p0", [H, BC], p0_d, F32)
        h0_t = cload("h0", [10, BC], h0_d, F32)
        id_t = cload("id32", [H, H], id_d, F32)
        idh_t = cload("id10", [10, 10], idh_d, F32)

        pp = ppp.tile([H, 128], F32, tag="pp", name="pp", space="PSUM")
        ppH = ppp.tile([10, 128], F32, tag="ppH", name="ppH", space="PSUM")

        rhs_tiles, lm_tiles, hm_tiles, u_tiles = [], [], [], []

        def pre(c):
            # chunk 0 splits each stream at slot 8 so the scan starts as
            # soon as the first slots land instead of waiting for the
            # whole-chunk transfers (readers of a tile wait on the full
            # DMA instruction that wrote it)
            cuts = (0, 8, TC) if c == 0 else (0, TC)
            r = rhsp.tile([K2, TC * 128], F16, tag="rhs")
            for a, b_ in zip(cuts[:-1], cuts[1:]):
                nc.sync.dma_start(
                    r[64:98, a * 128:b_ * 128],
                    rhsd_d[:, (c * TC + a) * 128:(c * TC + b_) * 128])
            rhs_tiles.append(r)
            m = lmp.tile([K2, TC * H], F16, tag="lm")
            for a, b_ in zip(cuts[:-1], cuts[1:]):
                nc.sync.dma_start(
                    m[:, a * H:b_ * H],
                    lm_d[:, (c * TC + a) * H:(c * TC + b_) * H])
            lm_tiles.append(m)
            hh = hmp.tile([KH, TC * 10], F16, tag="hm")
            for a, b_ in zip(cuts[:-1], cuts[1:]):
                nc.sync.dma_start(
                    hh[:, a * 10:b_ * 10],
                    hm_d[:, (c * TC + a) * 10:(c * TC + b_) * 10])
            hm_tiles.append(hh)
            u = usp.tile([10, TC * 128], F16, tag="u")
            u_tiles.append(u)

        def rslot(g):
            c, s = divmod(g, TC)
            return rhs_tiles[c][0:KM, 128 * s:128 * (s + 1)]

        def hslot(g):
            c, s = divmod(g, TC)
            return rhs_tiles[c][0:32, 128 * s:128 * (s + 1)]

        def lagslot(g):
            c, s = divmod(g, TC)
            return rhs_tiles[c][32:64, 128 * s:128 * (s + 1)]

        def hdslot(g):
            c, s = divmod(g, TC)
            return rhs_tiles[c][0:KH, 128 * s:128 * (s + 1)]

        def uslot(g):
            c, s = divmod(g, TC)
            return u_tiles[c][:, 128 * s:128 * (s + 1)]

        def lmsl(g):
            c, s = divmod(g, TC)
            return lm_tiles[c][0:KM, H * s:H * (s + 1)]

        def hmsl(g):
            c, s = divmod(g, TC)
            return hm_tiles[c][:, 10 * s:10 * (s + 1)]

        pre(0)
        nc.vector.memset(rhs_tiles[0][32:64, 0:128], 0.0)
        nc.tensor.matmul(pp[:], id_t[:], p0_t[:], start=True, stop=True,
                         skip_group_check=True)
        nc.tensor.matmul(ppH[:], idh_t[:], h0_t[:], start=True, stop=True,
                         skip_group_check=True)
        nc.vector.tensor_scalar_max(uslot(0), ppH[:], 0.0)

        for c in range(nchunks):
            if c + 1 < nchunks:
                pre(c + 1)
            u_t = u_tiles[c]
            po = pop.tile([128, 2 * TC], F32, tag="po", space="PSUM")

            for s in range(TC):
                g = c * TC + s
                if g < T - 1:
                    nc.scalar.activation(hslot(g), pp[:], AF.Tanh)
                    if g + 1 <= T - 2:
                        # replicate h_t into next slot's lag rows (off-chain)
                        nc.vector.tensor_copy(lagslot(g + 1), hslot(g))
                    # scan matmul first in the PE FIFO after TANH
                    nc.tensor.matmul(pp[:], lmsl(g), rslot(g),
                                     start=False, stop=True,
                                     skip_group_check=True)
                # head state update lags one step so it never delays MMm
                if 1 <= g:
                    nc.tensor.matmul(ppH[:], hmsl(g - 1), hdslot(g - 1),
                                     start=False, stop=True,
                                     skip_group_check=True)
                    nc.vector.tensor_scalar_max(uslot(g), ppH[:], 0.0)

                if s % 4 == 0 and s > 0:
                    b = s // 4 - 1
                    for k in range(4):
                        sl = 4 * b + k
                        nc.tensor.matmul(
                            po[:, 2 * sl:2 * sl + 2],
                            u_t[:, 128 * sl:128 * (sl + 1)], w2_t[:],
                            start=True, stop=True)


            b = TC // 4 - 1
            for k in range(4):
                sl = 4 * b + k
                nc.tensor.matmul(po[:, 2 * sl:2 * sl + 2],
                                 u_t[:, 128 * sl:128 * (sl + 1)], w2_t[:],
                                 start=True, stop=True)
            osb_t = osbp.tile([128, 2 * TC], F32, tag="osb")
            nc.vector.tensor_copy(osb_t[:], po[:])
            nc.sync.dma_start(out_d[:, 2 * c * TC:2 * (c + 1) * TC],
                              osb_t[:])

    nc.compile()
    return nc


def _prep_inputs_v6(x, t, y0, Wr1, br1, Wr2, br2, W1, b1, W2, b2, T_=T):
    f16, f32, f64 = np.float16, np.float32, np.float64
    x = np.asarray(x, f32)
    tt = np.asarray(t, f32)
    y0 = np.asarray(y0, f32)
    Wr1 = np.asarray(Wr1, f32)
    Wy, We = Wr1[:S], Wr1[S:]
    Wr2 = np.asarray(Wr2, f32)
    br1 = np.asarray(br1, f32)
    br2 = np.asarray(br2, f32)
    W1 = np.asarray(W1, f32)
    b1 = np.asarray(b1, f32)
    W2 = np.asarray(W2, f32)
    dt = np.diff(tt).astype(f32)

    # exact per-step weights in the 49-row basis, then split main/residual
    ex = np.zeros((T_, K_RHS, H), f64)
    Wt64 = f64(Wr2) @ f64(Wy)
    ex[:T_ - 1, 0:32] = dt[:, None, None].astype(f64) * Wt64[None]
    ex[:T_ - 1, 32:40] = f64(We)[None]
    ex[:T_ - 1, 40:48] = f64(We)[None]
    ex[:T_ - 1, 48] = dt[:, None].astype(f64) * (f64(br2) @ f64(Wy))[None]
    main = ex.astype(f16)
    resid = (ex - main.astype(f64)).astype(f16)

    # rows: 0:32 h | 32:64 h_lag | 64 ones | 65:73 d_c | 73:81 d_f
    #       | 81:89 d_c' | 89:97 d_f' | 97 ones'
    lm = np.zeros((T_, K2, H), f16)
    lm[:, 0:32] = main[:, 0:32]
    lm[1:, 32:64] = resid[:T_ - 1, 0:32]     # h-residual, lagged one step
    lm[:, 64] = main[:, 48]
    lm[:, 65:73] = main[:, 32:40]
    lm[:, 73:81] = main[:, 40:48]
    lm[:, 81:89] = resid[:, 32:40]
    lm[:, 89:97] = resid[:, 40:48]
    lm[:, 97] = resid[:, 48]

    M1 = f64(W1.T) @ np.linalg.pinv(f64(Wy.T))
    hm = np.zeros((T_, KH, 10), f64)
    hm[:, 0:32] = ex[:, 0:32] @ M1.T
    hm[:, 64] = ex[:, 48] @ M1.T
    hm = hm.astype(f16)

    lm_s = np.ascontiguousarray(lm.transpose(1, 0, 2).reshape(K2, T_ * H))
    hm_s = np.ascontiguousarray(hm.transpose(1, 0, 2).reshape(KH, T_ * 10))

    common = {
        "lm": lm_s, "hm": hm_s,
        "w2f": W2.astype(f16),
        "id32": np.eye(H, dtype=f32),
        "id10": np.eye(10, dtype=f32),
    }
    in_maps = []
    for k in range(NCORES):
        sl = slice(k * BC, (k + 1) * BC)
        eT = np.ascontiguousarray(x[sl].transpose(2, 1, 0))
        d = eT[:, 1:, :] - eT[:, :-1, :]
        d_c = d.astype(f16)
        d_f = (d - d_c.astype(f32)).astype(f16)
        rhsd = np.ones((34, T_, BC), f16)
        for base in (1, 17):                  # dest rows 65:81 and 81:97
            rhsd[base:base + 8, :T_ - 1] = d_c
            rhsd[base:base + 8, T_ - 1] = 0
            rhsd[base + 8:base + 16, :T_ - 1] = d_f
            rhsd[base + 8:base + 16, T_ - 1] = 0
        p0 = (f64(Wy.T) @ f64(y0[sl].T) + f64(We.T) @ f64(eT[:, 0, :])
              + f64(br1)[:, None]).astype(f32)
        pre10 = (M1 @ f64(p0) - M1 @ f64(We.T) @ f64(eT[:, 0, :])
                 - (M1 @ f64(br1))[:, None] + f64(b1)[:, None]).astype(f32)
        in_maps.append({
            "rhsd": rhsd.reshape(34, T_ * BC),
            "p0t": p0,
            "pre10": pre10,
            **common,
        })
    return in_maps


_NC_CACHE = {}


def kernel(x, t, y0, Wr1, br1, Wr2, br2, W1, b1, W2, b2):
    in_maps = _prep_inputs_v6(
        x, t, y0, Wr1, br1, Wr2, br2, W1, b1, W2, b2)
    key = ("v6g",)
    if key not in _NC_CACHE:
        _NC_CACHE[key] = build_ode_nc_v6(T=T, TC=64)
    nc = _NC_CACHE[key]
    res = bass_utils.run_bass_kernel_spmd(nc, in_maps,
                                          core_ids=list(range(NCORES)))
    outs = [res.results[k]["out"].reshape(BC, T, 2) for k in range(NCORES)]
    out = np.concatenate(outs, axis=0)
    b2 = np.asarray(b2, np.float32)
    if np.any(b2 != 0):
        out = out + b2[None, None, :]
    return out.astype(np.float32)

